# revision 6
# baseline (speedup 1.0000x reference)
"""Multi-head attention forward (B=8, S=1024, D=1024, H=16) on 8 trn2 NeuronCores.

Strategy: pure data parallelism — core c computes batch element c entirely
locally (no collectives). Per core:

  1. Transpose Q,K,V ([S,D] -> [D,S]) via PE-transpose (exact, fp32).
  2. Projections on PE in float32r (full-rate 4-byte matmul mode):
       qT[d,s] = Wq.T @ Q.T (+bq per-partition), kT likewise (+bk),
       v[s,d]  = V @ Wv  (laid out per-head with an appended ones column).
  3. Per head: scoresT[j,i] -> exp (ACT, scale=1/8 fused) -> PV matmul
     (v_aug.T @ expT) giving outT[d,i] rows 0..63 and the softmax row-sums
     in row 64 (from the ones column). OutCatT normalized by 1/rowsum via a
     PE outer-product broadcast + DVE multiply.
  4. Per head: scores[i,j] -> exp -> multiply by 1/rowsum (column form via
     tiny PE transposes of the rowsum rows) -> attn output (fp32 exact DMA).
  5. Final projection: out[i,e] = OutCatT.T @ Wo.

Biases: bq/bk applied on-device (per-partition adds in the transposed
projections). bv/bo are softmax-invariant / affine: out += bv @ Wo + bo is
added on host; attn does not depend on them. mask is guaranteed all-zero by
the input spec (fill=zeros) and softmax is shift-invariant in any case only
for row-constant shifts, so it is validated on host and must be zero.
"""

import sys

sys.path.insert(0, "/opt/trn_rl_repo")

import numpy as np

import concourse.bacc as bacc
import concourse.mybir as mybir
import concourse.tile as tile
from concourse.bass_utils import run_bass_kernel_spmd
from concourse.masks import make_identity

S = 1024
D = 1024
H = 16
DH = 64
P = 128
KT = D // P  # 8 contraction tiles
SBK = S // P  # 8 sequence blocks
F32 = mybir.dt.float32
F32R = mybir.dt.float32r

_CACHED_NC = None


def _build_nc():
    nc = bacc.Bacc("TRN2", target_bir_lowering=False, debug=False, num_devices=8)

    Qd = nc.dram_tensor("Qh", [S, D], F32, kind="ExternalInput")
    Kd = nc.dram_tensor("Kh", [S, D], F32, kind="ExternalInput")
    Vd = nc.dram_tensor("Vh", [S, D], F32, kind="ExternalInput")
    Wqd = nc.dram_tensor("Wq", [D, D], F32, kind="ExternalInput")
    Wkd = nc.dram_tensor("Wk", [D, D], F32, kind="ExternalInput")
    Wvd = nc.dram_tensor("Wv", [D, D], F32, kind="ExternalInput")
    Wod = nc.dram_tensor("Wo", [D, D], F32, kind="ExternalInput")
    bqd = nc.dram_tensor("bq", [D], F32, kind="ExternalInput")
    bkd = nc.dram_tensor("bk", [D], F32, kind="ExternalInput")
    attn_o = nc.dram_tensor("attn_o", [H, S, S], F32, kind="ExternalOutput")
    out_o = nc.dram_tensor("out_o", [S, D], F32, kind="ExternalOutput")

    with tile.TileContext(nc) as tc:
        _emit(nc, tc, Qd, Kd, Vd, Wqd, Wkd, Wvd, Wod, bqd, bkd, attn_o, out_o)

    if not nc.is_finalized():
        nc.finalize()
    return nc


def _transpose_into(nc, pools, Xd, XT_sb, ident):
    """DMA X [S,D] natural, PE-transpose 128x128 blocks into XT_sb [D,S] as
    [128, KT*1024] fp32r (k-tile t at free cols t*1024 + s)."""
    nat_pool, tps_pool = pools
    for sb_i in range(SBK):
        xnat = nat_pool.tile([P, D], F32, tag="xnat")
        nc.sync.dma_start(out=xnat, in_=Xd[sb_i * P : (sb_i + 1) * P, :])
        for kb in range(KT):
            pt = tps_pool.tile([P, P], F32, tag="tps")
            nc.tensor.transpose(pt, xnat[:, kb * P : (kb + 1) * P], ident)
            nc.vector.tensor_copy(
                XT_sb[:, kb * S + sb_i * P : kb * S + (sb_i + 1) * P], pt
            )


def _load_weight(nc, W_sb, Wd):
    for t in range(KT):
        nc.sync.dma_start(
            out=W_sb[:, t * D : (t + 1) * D],
            in_=Wd[t * P : (t + 1) * P, :].bitcast(F32R),
        )


def _emit(nc, tc, Qd, Kd, Vd, Wqd, Wkd, Wvd, Wod, bqd, bkd, attn_o, out_o):
    from contextlib import ExitStack

    VW = H * (DH + 1)  # 1040: per-head 64 data cols + ones col

    with ExitStack() as top:
        consts = top.enter_context(tc.tile_pool(name="consts", bufs=1))
        ident = consts.tile([P, P], F32)
        make_identity(nc, ident)
        bq_sb = consts.tile([P, KT], F32)
        bk_sb = consts.tile([P, KT], F32)
        nc.sync.dma_start(out=bq_sb, in_=bqd[:].rearrange("(t p) -> p t", p=P))
        nc.sync.dma_start(out=bk_sb, in_=bkd[:].rearrange("(t p) -> p t", p=P))
        ones_f = consts.tile([P, P], F32)
        nc.vector.memset(ones_f, 1.0)
        ones_col = consts.tile([1, P], F32R)
        nc.vector.tensor_copy(ones_col, ones_f[0:1, :])
        ones_cols128 = consts.tile([P, H], F32R)
        nc.vector.tensor_copy(ones_cols128, ones_f[:, 0:H])

        # persistent activations
        big = top.enter_context(tc.tile_pool(name="big", bufs=1))
        v_sb = big.tile([P, SBK * VW], F32R, tag="v")
        qT_sb = big.tile([P, KT * S], F32R, tag="qT")
        kT_sb = big.tile([P, KT * S], F32R, tag="kT")

        # ---- Phase V: v = V @ Wv, per-head layout + ones columns ----
        for sb_i in range(SBK):
            ones_dst = v_sb[:, sb_i * VW : (sb_i + 1) * VW].rearrange(
                "p (h c) -> p h c", c=DH + 1
            )[:, :, DH]
            nc.vector.tensor_copy(ones_dst, ones_cols128)
        with ExitStack() as ph:
            wp = ph.enter_context(tc.tile_pool(name="wv", bufs=1))
            xt = ph.enter_context(tc.tile_pool(name="vt", bufs=1))
            nat = ph.enter_context(tc.tile_pool(name="nat", bufs=3))
            tps = ph.enter_context(tc.tile_pool(name="tps", bufs=4, space="PSUM"))
            pps = ph.enter_context(tc.tile_pool(name="pps", bufs=2, space="PSUM"))
            Wv_sb = wp.tile([P, KT * D], F32R)
            _load_weight(nc, Wv_sb, Wvd)
            VT_sb = xt.tile([P, KT * S], F32R)
            _transpose_into(nc, (nat, tps), Vd, VT_sb, ident)
            for sb_i in range(SBK):
                pv = pps.tile([P, D], F32, tag="pp")
                for jn in range(2):
                    for t in range(KT):
                        nc.tensor.matmul(
                            pv[:, jn * 512 : (jn + 1) * 512],
                            VT_sb[:, t * S + sb_i * P : t * S + (sb_i + 1) * P],
                            Wv_sb[:, t * D + jn * 512 : t * D + (jn + 1) * 512],
                            start=(t == 0),
                            stop=(t == KT - 1),
                        )
                for h in range(H):
                    nc.vector.tensor_copy(
                        v_sb[
                            :,
                            sb_i * VW + h * (DH + 1) : sb_i * VW + h * (DH + 1) + DH,
                        ],
                        pv[:, h * DH : (h + 1) * DH],
                    )

        # ---- Phases Q, K: qT = Wq.T @ QT + bq, kT likewise ----
        for Xd, Wd, b_sb, dstT in ((Qd, Wqd, bq_sb, qT_sb), (Kd, Wkd, bk_sb, kT_sb)):
            with ExitStack() as ph:
                wp = ph.enter_context(tc.tile_pool(name="w", bufs=1))
                xt = ph.enter_context(tc.tile_pool(name="xt", bufs=1))
                nat = ph.enter_context(tc.tile_pool(name="nat", bufs=3))
                tps = ph.enter_context(tc.tile_pool(name="tps", bufs=4, space="PSUM"))
                pps = ph.enter_context(tc.tile_pool(name="pps", bufs=2, space="PSUM"))
                W_sb = wp.tile([P, KT * D], F32R)
                _load_weight(nc, W_sb, Wd)
                XT_sb = xt.tile([P, KT * S], F32R)
                _transpose_into(nc, (nat, tps), Xd, XT_sb, ident)
                for m in range(KT):
                    pq = pps.tile([P, S], F32, tag="pp")
                    for jn in range(2):
                        for t in range(KT):
                            nc.tensor.matmul(
                                pq[:, jn * 512 : (jn + 1) * 512],
                                W_sb[:, t * D + m * P : t * D + (m + 1) * P],
                                XT_sb[:, t * S + jn * 512 : t * S + (jn + 1) * 512],
                                start=(t == 0),
                                stop=(t == KT - 1),
                            )
                    nc.vector.tensor_scalar_add(
                        dstT[:, m * S : (m + 1) * S], pq, b_sb[:, m : m + 1]
                    )

        # ---- Stage II: per head, scoresT -> expT -> PV; normalize OutCatT ----
        big2 = top.enter_context(tc.tile_pool(name="big2", bufs=1))
        outcT = big2.tile([P, KT * S], F32R, tag="outcT")
        with ExitStack() as ph:
            sps = ph.enter_context(tc.tile_pool(name="sps", bufs=2, space="PSUM"))
            ops = ph.enter_context(tc.tile_pool(name="ops", bufs=2, space="PSUM"))
            pexp = ph.enter_context(tc.tile_pool(name="pexp", bufs=3))
            pbc = ph.enter_context(tc.tile_pool(name="pbc", bufs=2))
            for h in range(H):
                hp = (h % 2) * DH
                hc = (h // 2) * S
                po = ops.tile([DH + 1, S], F32, tag="po")
                for jb in range(SBK):
                    ps_t = sps.tile([P, S], F32, tag="sps")
                    for jn in range(2):
                        nc.tensor.matmul(
                            ps_t[:, jn * 512 : (jn + 1) * 512],
                            kT_sb[hp : hp + DH, hc + jb * P : hc + (jb + 1) * P],
                            qT_sb[hp : hp + DH, hc + jn * 512 : hc + (jn + 1) * 512],
                            start=True,
                            stop=True,
                        )
                    et = pexp.tile([P, S], F32R, tag="expT")
                    nc.scalar.activation(
                        et, ps_t, mybir.ActivationFunctionType.Exp, scale=0.125
                    )
                    for jn in range(2):
                        nc.tensor.matmul(
                            po[:, jn * 512 : (jn + 1) * 512],
                            v_sb[
                                :,
                                jb * VW + h * (DH + 1) : jb * VW + (h + 1) * (DH + 1),
                            ],
                            et[:, jn * 512 : (jn + 1) * 512],
                            start=(jb == 0),
                            stop=(jb == SBK - 1),
                        )
                # rowsums -> reciprocal (row form, partition 0) + fp32r copy
                rrf = pbc.tile([1, S], F32, tag="rrf")
                nc.vector.reciprocal(rrf, po[DH : DH + 1, :])
                rr = pbc.tile([1, S], F32R, tag="rr")
                nc.vector.tensor_copy(rr, rrf)
                pb = sps.tile([DH, S], F32, tag="sps")
                for jn in range(2):
                    nc.tensor.matmul(
                        pb[:, jn * 512 : (jn + 1) * 512],
                        ones_col[0:1, 0:DH],
                        rr[0:1, jn * 512 : (jn + 1) * 512],
                        start=True,
                        stop=True,
                    )
                bc = pbc.tile([DH, S], F32, tag="bc")
                nc.vector.tensor_copy(bc, pb)
                nc.vector.tensor_mul(
                    outcT[hp : hp + DH, hc : hc + S], po[0:DH, :], bc
                )

        # ---- Stage III (attn out) + Stage IV (final projection) ----
        with ExitStack() as ph:
            s3p = ph.enter_context(tc.tile_pool(name="s3p", bufs=2, space="PSUM"))
            s4p = ph.enter_context(tc.tile_pool(name="s4p", bufs=2, space="PSUM"))
            pattn = ph.enter_context(tc.tile_pool(name="pattn", bufs=3))
            pout = ph.enter_context(tc.tile_pool(name="pout", bufs=2))
            wo_p = ph.enter_context(tc.tile_pool(name="wo", bufs=1))
            Wo_sb = wo_p.tile([P, KT * D], F32R)
            _load_weight(nc, Wo_sb, Wod)

            for h in range(H):
                hp = (h % 2) * DH
                hc = (h // 2) * S
                for ib in range(SBK):
                    ps_t = s3p.tile([P, S], F32, tag="s3")
                    for jn in range(2):
                        nc.tensor.matmul(
                            ps_t[:, jn * 512 : (jn + 1) * 512],
                            qT_sb[hp : hp + DH, hc + ib * P : hc + (ib + 1) * P],
                            kT_sb[hp : hp + DH, hc + jn * 512 : hc + (jn + 1) * 512],
                            start=True,
                            stop=True,
                        )
                    at = pattn.tile([P, S], F32, tag="attn")
                    rsum = pattn.tile([P, 1], F32, tag="rsum")
                    nc.scalar.activation(
                        at,
                        ps_t,
                        mybir.ActivationFunctionType.Exp,
                        scale=0.125,
                        accum_out=rsum,
                    )
                    rcol = pattn.tile([P, 1], F32, tag="rcol")
                    nc.vector.reciprocal(rcol, rsum)
                    nc.vector.tensor_scalar_mul(at, at, rcol)
                    nc.sync.dma_start(
                        out=attn_o[h, ib * P : (ib + 1) * P, :], in_=at
                    )

            for ib in range(SBK):
                pf = s4p.tile([P, D], F32, tag="s4")
                for jn in range(2):
                    for t in range(KT):
                        nc.tensor.matmul(
                            pf[:, jn * 512 : (jn + 1) * 512],
                            outcT[:, t * S + ib * P : t * S + (ib + 1) * P],
                            Wo_sb[:, t * D + jn * 512 : t * D + (jn + 1) * 512],
                            start=(t == 0),
                            stop=(t == KT - 1),
                        )
                ot = pout.tile([P, D], F32, tag="ot")
                nc.vector.tensor_copy(ot, pf)
                nc.sync.dma_start(out=out_o[ib * P : (ib + 1) * P, :], in_=ot)


def _get_nc():
    global _CACHED_NC
    if _CACHED_NC is None:
        _CACHED_NC = _build_nc()
    return _CACHED_NC


def kernel(Q, K, V, mask, Wq, bq, Wk, bk, Wv, bv, Wo, bo, _want_results=False):
    Q = np.ascontiguousarray(np.asarray(Q, dtype=np.float32))
    K = np.ascontiguousarray(np.asarray(K, dtype=np.float32))
    V = np.ascontiguousarray(np.asarray(V, dtype=np.float32))
    Wq = np.ascontiguousarray(np.asarray(Wq, dtype=np.float32))
    Wk = np.ascontiguousarray(np.asarray(Wk, dtype=np.float32))
    Wv = np.ascontiguousarray(np.asarray(Wv, dtype=np.float32))
    Wo = np.ascontiguousarray(np.asarray(Wo, dtype=np.float32))
    bq = np.ascontiguousarray(np.asarray(bq, dtype=np.float32))
    bk = np.ascontiguousarray(np.asarray(bk, dtype=np.float32))
    bv = np.asarray(bv, dtype=np.float32)
    bo = np.asarray(bo, dtype=np.float32)
    mask = np.asarray(mask, dtype=np.float32)
    assert not np.any(mask), "kernel assumes zero mask (input spec fill=zeros)"

    nc = _get_nc()
    B = Q.shape[0]
    in_maps = [
        {
            "Qh": Q[c],
            "Kh": K[c],
            "Vh": V[c],
            "Wq": Wq,
            "Wk": Wk,
            "Wv": Wv,
            "Wo": Wo,
            "bq": bq,
            "bk": bk,
        }
        for c in range(B)
    ]
    res = run_bass_kernel_spmd(nc, in_maps, core_ids=list(range(B)))

    out = np.stack([res.results[c]["out_o"] for c in range(B)])
    attn = np.stack([res.results[c]["attn_o"] for c in range(B)])
    # host-side affine tail: v-bias and output bias (exact; zero per spec)
    out += (bv @ Wo + bo)[None, None, :]
    if _want_results:
        return (out, attn), res
    return out, attn


# revision 9
# speedup vs baseline: 1.1472x; 1.1472x over previous
"""Multi-head attention forward (B=8, S=1024, D=1024, H=16) on 8 trn2 NeuronCores.

Pure data parallelism: core c computes batch element c (no collectives).

Per core (fp16 matmul operands, fp32 PSUM accumulation, fp32 softmax):
  1. Q,K,V loaded with fp32->fp16 DMA cast, PE-transposed to [D,S].
  2. Projections (PE, fp16 @ 1cyc/row): qT[d,s] (+bq), kT[d,s] (+bk),
     v[s,d] per-head layout with an appended ones column.
  3. Per head pair (tile_position row packing, K=64): scoresT[j,i] -> exp
     (ACT, scale=1/8, fp16 out) -> PV matmul v_aug.T @ expT accumulating
     outT[d,i] + softmax rowsums (row 64, from the ones column). OutCatT
     normalized by 1/rowsum via PE outer-product broadcast + DVE multiply.
  4. Per head pair: scores[i,j] -> exp (fp32 out + accum_out rowsum) ->
     multiply by 1/rowsum -> attn (exact fp32 path for the attn output).
  5. Final projection out[i,e] = OutCatT.T @ Wo (fp16).

Biases: bq/bk on-device (per-partition adds); bv/bo are affine tails added
on host (out += bv @ Wo + bo, exact since softmax rows sum to 1); attn does
not depend on them. mask is asserted zero (input spec fill=zeros).
"""

import sys

sys.path.insert(0, "/opt/trn_rl_repo")

import numpy as np

import concourse.bacc as bacc
import concourse.mybir as mybir
import concourse.tile as tile
from concourse.bass_utils import run_bass_kernel_spmd
from concourse.masks import make_identity

S = 1024
D = 1024
H = 16
DH = 64
P = 128
KT = D // P  # 8
SBK = S // P  # 8
VW = H * (DH + 1)  # 1040
F32 = mybir.dt.float32
F16 = mybir.dt.float16

_CACHED_NC = None


def _build_nc():
    nc = bacc.Bacc("TRN2", target_bir_lowering=False, debug=False, num_devices=8)

    Qd = nc.dram_tensor("Qh", [S, D], F32, kind="ExternalInput")
    Kd = nc.dram_tensor("Kh", [S, D], F32, kind="ExternalInput")
    Vd = nc.dram_tensor("Vh", [S, D], F32, kind="ExternalInput")
    Wqd = nc.dram_tensor("Wq", [D, D], F32, kind="ExternalInput")
    Wkd = nc.dram_tensor("Wk", [D, D], F32, kind="ExternalInput")
    Wvd = nc.dram_tensor("Wv", [D, D], F32, kind="ExternalInput")
    Wod = nc.dram_tensor("Wo", [D, D], F32, kind="ExternalInput")
    bqd = nc.dram_tensor("bq", [D], F32, kind="ExternalInput")
    bkd = nc.dram_tensor("bk", [D], F32, kind="ExternalInput")
    attn_o = nc.dram_tensor("attn_o", [H, S, S], F32, kind="ExternalOutput")
    out_o = nc.dram_tensor("out_o", [S, D], F32, kind="ExternalOutput")

    with tile.TileContext(nc) as tc:
        _emit(nc, tc, Qd, Kd, Vd, Wqd, Wkd, Wvd, Wod, bqd, bkd, attn_o, out_o)

    if not nc.is_finalized():
        nc.finalize()
    return nc


def _transpose_into(nc, pools, Xd, XT_sb, ident16):
    """DMA-cast X [S,D] fp32->fp16, PE-transpose 128x128 blocks into
    XT_sb [128, KT*S] fp16 (k-tile t at free cols t*S + s)."""
    nat_pool, tps_pool = pools
    for sb_i in range(SBK):
        xnat = nat_pool.tile([P, D], F16, tag="xnat")
        nc.gpsimd.dma_start(out=xnat, in_=Xd[sb_i * P : (sb_i + 1) * P, :])
        for kb in range(KT):
            pt = tps_pool.tile([P, P], F16, tag="tps")
            nc.tensor.transpose(pt, xnat[:, kb * P : (kb + 1) * P], ident16)
            nc.vector.tensor_copy(
                XT_sb[:, kb * S + sb_i * P : kb * S + (sb_i + 1) * P], pt
            )


def _load_weight(nc, W_sb, Wd):
    for t in range(KT):
        nc.gpsimd.dma_start(
            out=W_sb[:, t * D : (t + 1) * D], in_=Wd[t * P : (t + 1) * P, :]
        )


def _emit(nc, tc, Qd, Kd, Vd, Wqd, Wkd, Wvd, Wod, bqd, bkd, attn_o, out_o):
    from contextlib import ExitStack

    with ExitStack() as top:
        consts = top.enter_context(tc.tile_pool(name="consts", bufs=1))
        ones_f = consts.tile([P, P], F32)
        nc.vector.memset(ones_f, 1.0)
        ident16 = consts.tile([P, P], F16)
        make_identity(nc, ident16)
        bq_sb = consts.tile([P, KT], F32)
        bk_sb = consts.tile([P, KT], F32)
        nc.sync.dma_start(out=bq_sb, in_=bqd[:].rearrange("(t p) -> p t", p=P))
        nc.sync.dma_start(out=bk_sb, in_=bkd[:].rearrange("(t p) -> p t", p=P))
        ones_col = consts.tile([1, P], F16)
        nc.vector.tensor_copy(ones_col, ones_f[0:1, :])
        ones_cols128 = consts.tile([P, H], F16)
        nc.vector.tensor_copy(ones_cols128, ones_f[:, 0:H])

        big = top.enter_context(tc.tile_pool(name="big", bufs=1))
        v_sb = big.tile([P, SBK * VW], F16, tag="v")
        qT_sb = big.tile([P, KT * S], F16, tag="qT")
        kT_sb = big.tile([P, KT * S], F16, tag="kT")

        # ---- Phase V ----
        for sb_i in range(SBK):
            ones_dst = v_sb[:, sb_i * VW : (sb_i + 1) * VW].rearrange(
                "p (h c) -> p h c", c=DH + 1
            )[:, :, DH]
            nc.vector.tensor_copy(ones_dst, ones_cols128)
        with ExitStack() as ph:
            wp = ph.enter_context(tc.tile_pool(name="wv", bufs=1))
            xt = ph.enter_context(tc.tile_pool(name="vt", bufs=1))
            nat = ph.enter_context(tc.tile_pool(name="nat", bufs=3))
            tps = ph.enter_context(tc.tile_pool(name="tps", bufs=4, space="PSUM"))
            pps = ph.enter_context(tc.tile_pool(name="pps", bufs=2, space="PSUM"))
            Wv_sb = wp.tile([P, KT * D], F16)
            _load_weight(nc, Wv_sb, Wvd)
            VT_sb = xt.tile([P, KT * S], F16)
            _transpose_into(nc, (nat, tps), Vd, VT_sb, ident16)
            for sb_i in range(SBK):
                pv = pps.tile([P, D], F32, tag="pp")
                for jn in range(2):
                    for t in range(KT):
                        nc.tensor.matmul(
                            pv[:, jn * 512 : (jn + 1) * 512],
                            VT_sb[:, t * S + sb_i * P : t * S + (sb_i + 1) * P],
                            Wv_sb[:, t * D + jn * 512 : t * D + (jn + 1) * 512],
                            start=(t == 0),
                            stop=(t == KT - 1),
                        )
                for h in range(H):
                    nc.vector.tensor_copy(
                        v_sb[
                            :,
                            sb_i * VW + h * (DH + 1) : sb_i * VW + h * (DH + 1) + DH,
                        ],
                        pv[:, h * DH : (h + 1) * DH],
                    )

        # ---- Phases Q, K ----
        for Xd, Wd, b_sb, dstT in ((Qd, Wqd, bq_sb, qT_sb), (Kd, Wkd, bk_sb, kT_sb)):
            with ExitStack() as ph:
                wp = ph.enter_context(tc.tile_pool(name="w", bufs=1))
                xt = ph.enter_context(tc.tile_pool(name="xt", bufs=1))
                nat = ph.enter_context(tc.tile_pool(name="nat", bufs=3))
                tps = ph.enter_context(tc.tile_pool(name="tps", bufs=4, space="PSUM"))
                pps = ph.enter_context(tc.tile_pool(name="pps", bufs=2, space="PSUM"))
                W_sb = wp.tile([P, KT * D], F16)
                _load_weight(nc, W_sb, Wd)
                XT_sb = xt.tile([P, KT * S], F16)
                _transpose_into(nc, (nat, tps), Xd, XT_sb, ident16)
                for m in range(KT):
                    pq = pps.tile([P, S], F32, tag="pp")
                    for jn in range(2):
                        for t in range(KT):
                            nc.tensor.matmul(
                                pq[:, jn * 512 : (jn + 1) * 512],
                                W_sb[:, t * D + m * P : t * D + (m + 1) * P],
                                XT_sb[:, t * S + jn * 512 : t * S + (jn + 1) * 512],
                                start=(t == 0),
                                stop=(t == KT - 1),
                            )
                    nc.vector.tensor_scalar_add(
                        dstT[:, m * S : (m + 1) * S], pq, b_sb[:, m : m + 1]
                    )

        # ---- Stage II: head pairs, packed scoresT -> expT -> PV ----
        big2 = top.enter_context(tc.tile_pool(name="big2", bufs=1))
        outcT = big2.tile([P, KT * S], F16, tag="outcT")
        with ExitStack() as ph:
            sps = ph.enter_context(tc.tile_pool(name="sps", bufs=2, space="PSUM"))
            ops = ph.enter_context(tc.tile_pool(name="ops", bufs=2, space="PSUM"))
            pexp = ph.enter_context(tc.tile_pool(name="pexp", bufs=4))
            pbc = ph.enter_context(tc.tile_pool(name="pbc", bufs=2))
            for pr in range(H // 2):
                hc = pr * S
                po = [ops.tile([DH + 1, S], F32, tag="po", name=f"po{_u}") for _u in range(2)]
                for jb in range(SBK):
                    pst = [sps.tile([P, S], F32, tag="sps", name=f"pst{_u}") for _u in range(2)]
                    for u in range(2):
                        hp = u * DH
                        for jn in range(2):
                            nc.tensor.matmul(
                                pst[u][:, jn * 512 : (jn + 1) * 512],
                                kT_sb[hp : hp + DH, hc + jb * P : hc + (jb + 1) * P],
                                qT_sb[
                                    hp : hp + DH, hc + jn * 512 : hc + (jn + 1) * 512
                                ],
                                start=True,
                                stop=True,
                                tile_position=(hp, 0),
                            )
                    for u in range(2):
                        h = 2 * pr + u
                        et = pexp.tile([P, S], F16, tag="expT")
                        nc.scalar.activation(
                            et, pst[u], mybir.ActivationFunctionType.Exp, scale=0.125
                        )
                        for jn in range(2):
                            nc.tensor.matmul(
                                po[u][:, jn * 512 : (jn + 1) * 512],
                                v_sb[
                                    :,
                                    jb * VW
                                    + h * (DH + 1) : jb * VW
                                    + (h + 1) * (DH + 1),
                                ],
                                et[:, jn * 512 : (jn + 1) * 512],
                                start=(jb == 0),
                                stop=(jb == SBK - 1),
                            )
                for u in range(2):
                    hp = u * DH
                    rrf = pbc.tile([1, S], F32, tag="rrf")
                    nc.vector.reciprocal(rrf, po[u][DH : DH + 1, :])
                    rr = pbc.tile([1, S], F16, tag="rr")
                    nc.vector.tensor_copy(rr, rrf)
                    pb = sps.tile([DH, S], F32, tag="sps")
                    for jn in range(2):
                        nc.tensor.matmul(
                            pb[:, jn * 512 : (jn + 1) * 512],
                            ones_col[0:1, 0:DH],
                            rr[0:1, jn * 512 : (jn + 1) * 512],
                            start=True,
                            stop=True,
                        )
                    bc = pbc.tile([DH, S], F32, tag="bc")
                    nc.vector.tensor_copy(bc, pb)
                    nc.vector.tensor_mul(
                        outcT[hp : hp + DH, hc : hc + S], po[u][0:DH, :], bc
                    )

        # ---- Stage III (attn out, packed) + Stage IV (final projection) ----
        with ExitStack() as ph:
            s3p = ph.enter_context(tc.tile_pool(name="s3p", bufs=2, space="PSUM"))
            s4p = ph.enter_context(tc.tile_pool(name="s4p", bufs=2, space="PSUM"))
            pattn = ph.enter_context(tc.tile_pool(name="pattn", bufs=4))
            pout = ph.enter_context(tc.tile_pool(name="pout", bufs=2))
            wo_p = ph.enter_context(tc.tile_pool(name="wo", bufs=1))
            Wo_sb = wo_p.tile([P, KT * D], F16)
            _load_weight(nc, Wo_sb, Wod)

            for pr in range(H // 2):
                hc = pr * S
                for ib in range(SBK):
                    pst = [s3p.tile([P, S], F32, tag="s3", name=f"pst3_{_u}") for _u in range(2)]
                    for u in range(2):
                        hp = u * DH
                        for jn in range(2):
                            nc.tensor.matmul(
                                pst[u][:, jn * 512 : (jn + 1) * 512],
                                qT_sb[hp : hp + DH, hc + ib * P : hc + (ib + 1) * P],
                                kT_sb[
                                    hp : hp + DH, hc + jn * 512 : hc + (jn + 1) * 512
                                ],
                                start=True,
                                stop=True,
                                tile_position=(hp, 0),
                            )
                    for u in range(2):
                        h = 2 * pr + u
                        at = pattn.tile([P, S], F32, tag="attn")
                        rsum = pattn.tile([P, 1], F32, tag="rsum")
                        nc.scalar.activation(
                            at,
                            pst[u],
                            mybir.ActivationFunctionType.Exp,
                            scale=0.125,
                            accum_out=rsum,
                        )
                        rcol = pattn.tile([P, 1], F32, tag="rcol")
                        nc.vector.reciprocal(rcol, rsum)
                        nc.vector.tensor_scalar_mul(at, at, rcol)
                        nc.sync.dma_start(
                            out=attn_o[h, ib * P : (ib + 1) * P, :], in_=at
                        )

            for ib in range(SBK):
                pf = s4p.tile([P, D], F32, tag="s4")
                for jn in range(2):
                    for t in range(KT):
                        nc.tensor.matmul(
                            pf[:, jn * 512 : (jn + 1) * 512],
                            outcT[:, t * S + ib * P : t * S + (ib + 1) * P],
                            Wo_sb[:, t * D + jn * 512 : t * D + (jn + 1) * 512],
                            start=(t == 0),
                            stop=(t == KT - 1),
                        )
                ot = pout.tile([P, D], F32, tag="ot")
                nc.vector.tensor_copy(ot, pf)
                nc.sync.dma_start(out=out_o[ib * P : (ib + 1) * P, :], in_=ot)


def _get_nc():
    global _CACHED_NC
    if _CACHED_NC is None:
        _CACHED_NC = _build_nc()
    return _CACHED_NC


def kernel(Q, K, V, mask, Wq, bq, Wk, bk, Wv, bv, Wo, bo, _want_results=False):
    Q = np.ascontiguousarray(np.asarray(Q, dtype=np.float32))
    K = np.ascontiguousarray(np.asarray(K, dtype=np.float32))
    V = np.ascontiguousarray(np.asarray(V, dtype=np.float32))
    Wq = np.ascontiguousarray(np.asarray(Wq, dtype=np.float32))
    Wk = np.ascontiguousarray(np.asarray(Wk, dtype=np.float32))
    Wv = np.ascontiguousarray(np.asarray(Wv, dtype=np.float32))
    Wo = np.ascontiguousarray(np.asarray(Wo, dtype=np.float32))
    bq = np.ascontiguousarray(np.asarray(bq, dtype=np.float32))
    bk = np.ascontiguousarray(np.asarray(bk, dtype=np.float32))
    bv = np.asarray(bv, dtype=np.float32)
    bo = np.asarray(bo, dtype=np.float32)
    mask = np.asarray(mask, dtype=np.float32)
    assert not np.any(mask), "kernel assumes zero mask (input spec fill=zeros)"

    nc = _get_nc()
    B = Q.shape[0]
    in_maps = [
        {
            "Qh": Q[c],
            "Kh": K[c],
            "Vh": V[c],
            "Wq": Wq,
            "Wk": Wk,
            "Wv": Wv,
            "Wo": Wo,
            "bq": bq,
            "bk": bk,
        }
        for c in range(B)
    ]
    res = run_bass_kernel_spmd(nc, in_maps, core_ids=list(range(B)))

    out = np.stack([res.results[c]["out_o"] for c in range(B)])
    attn = np.stack([res.results[c]["attn_o"] for c in range(B)])
    out += (bv @ Wo + bo)[None, None, :]
    if _want_results:
        return (out, attn), res
    return out, attn


# revision 11
# speedup vs baseline: 1.1673x; 1.0175x over previous
"""Multi-head attention forward (B=8, S=1024, D=1024, H=16) on 8 trn2 NeuronCores.

Pure data parallelism: core c computes batch element c (no collectives).

Per core (fp16 matmul operands, fp32 PSUM accumulation, fp32 softmax):
  1. Q,K,V loaded with fp32->fp16 DMA cast, PE-transposed to [D,S].
  2. Projections (PE, fp16 @ 1cyc/row): qT[d,s] (+bq), kT[d,s] (+bk),
     v[s,d] per-head layout with an appended ones column.
  3. Per head pair (tile_position row packing, K=64): scoresT[j,i] -> exp
     (ACT, scale=1/8, fp16 out) -> PV matmul v_aug.T @ expT accumulating
     outT[d,i] + softmax rowsums (row 64, from the ones column). OutCatT
     normalized by 1/rowsum via PE outer-product broadcast + DVE multiply.
  4. Per head pair: scores[i,j] -> exp (fp32 out + accum_out rowsum) ->
     multiply by 1/rowsum -> attn (exact fp32 path for the attn output).
  5. Final projection out[i,e] = OutCatT.T @ Wo (fp16).

Biases: bq/bk on-device (per-partition adds); bv/bo are affine tails added
on host (out += bv @ Wo + bo, exact since softmax rows sum to 1); attn does
not depend on them. mask is asserted zero (input spec fill=zeros).
"""

import sys

sys.path.insert(0, "/opt/trn_rl_repo")

import numpy as np

import concourse.bacc as bacc
import concourse.mybir as mybir
import concourse.tile as tile
from concourse.bass_utils import run_bass_kernel_spmd
from concourse.masks import make_identity

S = 1024
D = 1024
H = 16
DH = 64
P = 128
KT = D // P  # 8
SBK = S // P  # 8
VW = H * (DH + 1)  # 1040
F32 = mybir.dt.float32
F16 = mybir.dt.float16

_CACHED_NC = None


def _build_nc():
    nc = bacc.Bacc("TRN2", target_bir_lowering=False, debug=False, num_devices=8)

    Qd = nc.dram_tensor("Qh", [S, D], F32, kind="ExternalInput")
    Kd = nc.dram_tensor("Kh", [S, D], F32, kind="ExternalInput")
    Vd = nc.dram_tensor("Vh", [S, D], F32, kind="ExternalInput")
    Wqd = nc.dram_tensor("Wq", [D, D], F32, kind="ExternalInput")
    Wkd = nc.dram_tensor("Wk", [D, D], F32, kind="ExternalInput")
    Wvd = nc.dram_tensor("Wv", [D, D], F32, kind="ExternalInput")
    Wod = nc.dram_tensor("Wo", [D, D], F32, kind="ExternalInput")
    bqd = nc.dram_tensor("bq", [D], F32, kind="ExternalInput")
    bkd = nc.dram_tensor("bk", [D], F32, kind="ExternalInput")
    attn_o = nc.dram_tensor("attn_o", [H, S, S], F32, kind="ExternalOutput")
    out_o = nc.dram_tensor("out_o", [S, D], F32, kind="ExternalOutput")

    with tile.TileContext(nc) as tc:
        _emit(nc, tc, Qd, Kd, Vd, Wqd, Wkd, Wvd, Wod, bqd, bkd, attn_o, out_o)

    if not nc.is_finalized():
        nc.finalize()
    return nc


def _transpose_into(nc, pools, Xd, XT_sb, ident16):
    """DMA-cast X [S,D] fp32->fp16, PE-transpose 128x128 blocks into
    XT_sb [128, KT*S] fp16 (k-tile t at free cols t*S + s)."""
    nat_pool, tps_pool = pools
    for sb_i in range(SBK):
        xnat = nat_pool.tile([P, D], F16, tag="xnat")
        nc.gpsimd.dma_start(out=xnat, in_=Xd[sb_i * P : (sb_i + 1) * P, :])
        for kb in range(KT):
            pt = tps_pool.tile([P, P], F16, tag="tps")
            nc.tensor.transpose(pt, xnat[:, kb * P : (kb + 1) * P], ident16)
            nc.vector.tensor_copy(
                XT_sb[:, kb * S + sb_i * P : kb * S + (sb_i + 1) * P], pt
            )


def _load_weight(nc, W_sb, Wd):
    for t in range(KT):
        nc.gpsimd.dma_start(
            out=W_sb[:, t * D : (t + 1) * D], in_=Wd[t * P : (t + 1) * P, :]
        )


def _emit(nc, tc, Qd, Kd, Vd, Wqd, Wkd, Wvd, Wod, bqd, bkd, attn_o, out_o):
    from contextlib import ExitStack

    with ExitStack() as top:
        consts = top.enter_context(tc.tile_pool(name="consts", bufs=1))
        ones_f = consts.tile([P, P], F32)
        nc.vector.memset(ones_f, 1.0)
        ident16 = consts.tile([P, P], F16)
        make_identity(nc, ident16)
        bq_sb = consts.tile([P, KT], F32)
        bk_sb = consts.tile([P, KT], F32)
        nc.sync.dma_start(out=bq_sb, in_=bqd[:].rearrange("(t p) -> p t", p=P))
        nc.sync.dma_start(out=bk_sb, in_=bkd[:].rearrange("(t p) -> p t", p=P))
        ones_col = consts.tile([1, P], F16)
        nc.vector.tensor_copy(ones_col, ones_f[0:1, :])
        ones_cols128 = consts.tile([P, H], F16)
        nc.vector.tensor_copy(ones_cols128, ones_f[:, 0:H])

        big = top.enter_context(tc.tile_pool(name="big", bufs=1))
        v_sb = big.tile([P, SBK * VW], F16, tag="v")
        QT_sb = big.tile([P, KT * S], F16, tag="QT")
        KTr_sb = big.tile([P, KT * S], F16, tag="KTr")
        qT_sb = big.tile([P, KT * S], F16, tag="qT")
        kT_sb = big.tile([P, KT * S], F16, tag="kT")
        outcT = big.tile([P, KT * S], F16, tag="outcT")
        Wq_sb = big.tile([P, KT * D], F16, tag="Wq")
        Wk_sb = big.tile([P, KT * D], F16, tag="Wk")
        Wo_sb = big.tile([P, KT * D], F16, tag="Wo")
        rcols = big.tile([P, H * SBK], F32, tag="rcols")  # col h*SBK+ib

        _load_weight(nc, Wq_sb, Wqd)
        _load_weight(nc, Wk_sb, Wkd)
        _load_weight(nc, Wo_sb, Wod)

        # ---- Prefix: V phase (VT + v), QT/KT transposes ----
        for sb_i in range(SBK):
            ones_dst = v_sb[:, sb_i * VW : (sb_i + 1) * VW].rearrange(
                "p (h c) -> p h c", c=DH + 1
            )[:, :, DH]
            nc.vector.tensor_copy(ones_dst, ones_cols128)
        with ExitStack() as ph:
            wp = ph.enter_context(tc.tile_pool(name="wv", bufs=1))
            xt = ph.enter_context(tc.tile_pool(name="vt", bufs=1))
            nat = ph.enter_context(tc.tile_pool(name="nat", bufs=3))
            tps = ph.enter_context(tc.tile_pool(name="tps", bufs=4, space="PSUM"))
            pps = ph.enter_context(tc.tile_pool(name="pps", bufs=2, space="PSUM"))
            Wv_sb = wp.tile([P, KT * D], F16)
            _load_weight(nc, Wv_sb, Wvd)
            VT_sb = xt.tile([P, KT * S], F16)
            _transpose_into(nc, (nat, tps), Vd, VT_sb, ident16)
            for sb_i in range(SBK):
                pv = pps.tile([P, D], F32, tag="pp")
                for jn in range(2):
                    for t in range(KT):
                        nc.tensor.matmul(
                            pv[:, jn * 512 : (jn + 1) * 512],
                            VT_sb[:, t * S + sb_i * P : t * S + (sb_i + 1) * P],
                            Wv_sb[:, t * D + jn * 512 : t * D + (jn + 1) * 512],
                            start=(t == 0),
                            stop=(t == KT - 1),
                        )
                for h in range(H):
                    nc.vector.tensor_copy(
                        v_sb[
                            :,
                            sb_i * VW + h * (DH + 1) : sb_i * VW + h * (DH + 1) + DH,
                        ],
                        pv[:, h * DH : (h + 1) * DH],
                    )
            _transpose_into(nc, (nat, tps), Qd, QT_sb, ident16)
            _transpose_into(nc, (nat, tps), Kd, KTr_sb, ident16)

        # ---- Main pipeline: per head pair, projections + scoresT/expT/PV
        #      interleaved with previous pair's scores/exp/attn ----
        mp = top.enter_context(tc.tile_pool(name="mp", bufs=2, space="PSUM"))
        ops = top.enter_context(tc.tile_pool(name="ops", bufs=2, space="PSUM"))
        pexp = top.enter_context(tc.tile_pool(name="pexp", bufs=4))
        pbc = top.enter_context(tc.tile_pool(name="pbc", bufs=2))
        pattn = top.enter_context(tc.tile_pool(name="pattn", bufs=4))
        pout = top.enter_context(tc.tile_pool(name="pout", bufs=2))

        def emit_proj(m):
            # qT/kT block m from QT/KT (contraction over all KT k-tiles)
            for W_sb, XT, b_sb, dstT in (
                (Wq_sb, QT_sb, bq_sb, qT_sb),
                (Wk_sb, KTr_sb, bk_sb, kT_sb),
            ):
                pq = mp.tile([P, S], F32, tag="sc", name=f"proj{m}")
                for jn in range(2):
                    for t in range(KT):
                        nc.tensor.matmul(
                            pq[:, jn * 512 : (jn + 1) * 512],
                            W_sb[:, t * D + m * P : t * D + (m + 1) * P],
                            XT[:, t * S + jn * 512 : t * S + (jn + 1) * 512],
                            start=(t == 0),
                            stop=(t == KT - 1),
                        )
                nc.vector.tensor_scalar_add(
                    dstT[:, m * S : (m + 1) * S], pq, b_sb[:, m : m + 1]
                )

        def emit_stage2_step(pr, jb, po):
            hc = pr * S
            pst = [
                mp.tile([P, S], F32, tag="sc", name=f"s2_{pr}_{jb}_{_u}")
                for _u in range(2)
            ]
            for u in range(2):
                hp = u * DH
                for jn in range(2):
                    nc.tensor.matmul(
                        pst[u][:, jn * 512 : (jn + 1) * 512],
                        kT_sb[hp : hp + DH, hc + jb * P : hc + (jb + 1) * P],
                        qT_sb[hp : hp + DH, hc + jn * 512 : hc + (jn + 1) * 512],
                        start=True,
                        stop=True,
                        tile_position=(hp, 0),
                    )
            for u in range(2):
                h = 2 * pr + u
                et = pexp.tile([P, S], F16, tag="expT")
                nc.scalar.activation(
                    et, pst[u], mybir.ActivationFunctionType.Exp, scale=0.125
                )
                for jn in range(2):
                    nc.tensor.matmul(
                        po[u][:, jn * 512 : (jn + 1) * 512],
                        v_sb[
                            :,
                            jb * VW + h * (DH + 1) : jb * VW + (h + 1) * (DH + 1),
                        ],
                        et[:, jn * 512 : (jn + 1) * 512],
                        start=(jb == 0),
                        stop=(jb == SBK - 1),
                    )

        def emit_pair_end(pr, po):
            # rowsums -> reciprocal rows -> (a) fp16 bcast for outcT norm,
            # (b) tiny PE transposes into column form for the attn path
            hc = pr * S
            for u in range(2):
                h = 2 * pr + u
                hp = u * DH
                rrf = pbc.tile([1, S], F32, tag="rrf")
                nc.vector.reciprocal(rrf, po[u][DH : DH + 1, :])
                rr = pbc.tile([1, S], F16, tag="rr")
                nc.vector.tensor_copy(rr, rrf)
                pb = mp.tile([DH, S], F32, tag="sc", name=f"bc{pr}_{u}")
                for jn in range(2):
                    nc.tensor.matmul(
                        pb[:, jn * 512 : (jn + 1) * 512],
                        ones_col[0:1, 0:DH],
                        rr[0:1, jn * 512 : (jn + 1) * 512],
                        start=True,
                        stop=True,
                    )
                bc = pbc.tile([DH, S], F32, tag="bc")
                nc.vector.tensor_copy(bc, pb)
                nc.vector.tensor_mul(
                    outcT[hp : hp + DH, hc : hc + S], po[u][0:DH, :], bc
                )
                prc = mp.tile([P, SBK], F32, tag="sc", name=f"rc{pr}_{u}")
                for b in range(SBK):
                    nc.tensor.transpose(
                        prc[:, b : b + 1],
                        rrf[0:1, b * P : (b + 1) * P],
                        ones_f[0:1, 0:1],
                    )
                nc.vector.tensor_copy(
                    rcols[:, h * SBK : (h + 1) * SBK], prc
                )

        def emit_stage3_step(pr, ib):
            hc = pr * S
            pst = [
                mp.tile([P, S], F32, tag="sc", name=f"s3_{pr}_{ib}_{_u}")
                for _u in range(2)
            ]
            for u in range(2):
                hp = u * DH
                for jn in range(2):
                    nc.tensor.matmul(
                        pst[u][:, jn * 512 : (jn + 1) * 512],
                        qT_sb[hp : hp + DH, hc + ib * P : hc + (ib + 1) * P],
                        kT_sb[hp : hp + DH, hc + jn * 512 : hc + (jn + 1) * 512],
                        start=True,
                        stop=True,
                        tile_position=(hp, 0),
                    )
            for u in range(2):
                h = 2 * pr + u
                at = pattn.tile([P, S], F32, tag="attn")
                nc.scalar.activation(
                    at, pst[u], mybir.ActivationFunctionType.Exp, scale=0.125
                )
                nc.vector.tensor_scalar_mul(
                    at, at, rcols[:, h * SBK + ib : h * SBK + ib + 1]
                )
                nc.sync.dma_start(out=attn_o[h, ib * P : (ib + 1) * P, :], in_=at)

        emit_proj(0)
        po_by_pair = {}
        for pr in range(H // 2 + 1):
            if pr < H // 2:
                if pr + 1 < H // 2:
                    emit_proj(pr + 1)
                po = [
                    ops.tile([DH + 1, S], F32, tag="po", name=f"po{pr}_{_u}")
                    for _u in range(2)
                ]
                po_by_pair[pr] = po
            for step in range(SBK):
                if pr < H // 2:
                    emit_stage2_step(pr, step, po_by_pair[pr])
                if pr > 0:
                    emit_stage3_step(pr - 1, step)
            if pr < H // 2:
                emit_pair_end(pr, po_by_pair[pr])
                del po_by_pair[pr]

        # ---- Stage IV: final projection ----
        for ib in range(SBK):
            pf = mp.tile([P, D], F32, tag="sc", name=f"fin{ib}")
            for jn in range(2):
                for t in range(KT):
                    nc.tensor.matmul(
                        pf[:, jn * 512 : (jn + 1) * 512],
                        outcT[:, t * S + ib * P : t * S + (ib + 1) * P],
                        Wo_sb[:, t * D + jn * 512 : t * D + (jn + 1) * 512],
                        start=(t == 0),
                        stop=(t == KT - 1),
                    )
            ot = pout.tile([P, D], F32, tag="ot")
            nc.vector.tensor_copy(ot, pf)
            nc.sync.dma_start(out=out_o[ib * P : (ib + 1) * P, :], in_=ot)


def _get_nc():
    global _CACHED_NC
    if _CACHED_NC is None:
        _CACHED_NC = _build_nc()
    return _CACHED_NC


def kernel(Q, K, V, mask, Wq, bq, Wk, bk, Wv, bv, Wo, bo, _want_results=False):
    Q = np.ascontiguousarray(np.asarray(Q, dtype=np.float32))
    K = np.ascontiguousarray(np.asarray(K, dtype=np.float32))
    V = np.ascontiguousarray(np.asarray(V, dtype=np.float32))
    Wq = np.ascontiguousarray(np.asarray(Wq, dtype=np.float32))
    Wk = np.ascontiguousarray(np.asarray(Wk, dtype=np.float32))
    Wv = np.ascontiguousarray(np.asarray(Wv, dtype=np.float32))
    Wo = np.ascontiguousarray(np.asarray(Wo, dtype=np.float32))
    bq = np.ascontiguousarray(np.asarray(bq, dtype=np.float32))
    bk = np.ascontiguousarray(np.asarray(bk, dtype=np.float32))
    bv = np.asarray(bv, dtype=np.float32)
    bo = np.asarray(bo, dtype=np.float32)
    mask = np.asarray(mask, dtype=np.float32)
    assert not np.any(mask), "kernel assumes zero mask (input spec fill=zeros)"

    nc = _get_nc()
    B = Q.shape[0]
    in_maps = [
        {
            "Qh": Q[c],
            "Kh": K[c],
            "Vh": V[c],
            "Wq": Wq,
            "Wk": Wk,
            "Wv": Wv,
            "Wo": Wo,
            "bq": bq,
            "bk": bk,
        }
        for c in range(B)
    ]
    res = run_bass_kernel_spmd(nc, in_maps, core_ids=list(range(B)))

    out = np.stack([res.results[c]["out_o"] for c in range(B)])
    attn = np.stack([res.results[c]["attn_o"] for c in range(B)])
    out += (bv @ Wo + bo)[None, None, :]
    if _want_results:
        return (out, attn), res
    return out, attn


# revision 13
# speedup vs baseline: 1.4411x; 1.2346x over previous
"""Multi-head attention forward (B=8, S=1024, D=1024, H=16) on 8 trn2 NeuronCores.

Pure data parallelism: core c computes batch element c (no collectives).

Per core (fp16 matmul operands, fp32 PSUM accumulation, fp32 softmax):
  1. Q,K,V loaded with fp32->fp16 DMA cast, PE-transposed to [D,S].
  2. Projections (PE, fp16 @ 1cyc/row): qT[d,s] (+bq), kT[d,s] (+bk),
     v[s,d] per-head layout with an appended ones column.
  3. Per head pair (tile_position row packing, K=64): scoresT[j,i] -> exp
     (ACT, scale=1/8, fp16 out) -> PV matmul v_aug.T @ expT accumulating
     outT[d,i] + softmax rowsums (row 64, from the ones column). OutCatT
     normalized by 1/rowsum via PE outer-product broadcast + DVE multiply.
  4. Per head pair: scores[i,j] -> exp (fp32 out + accum_out rowsum) ->
     multiply by 1/rowsum -> attn (exact fp32 path for the attn output).
  5. Final projection out[i,e] = OutCatT.T @ Wo (fp16).

Biases: bq/bk on-device (per-partition adds); bv/bo are affine tails added
on host (out += bv @ Wo + bo, exact since softmax rows sum to 1); attn does
not depend on them. mask is asserted zero (input spec fill=zeros).
"""

import sys

sys.path.insert(0, "/opt/trn_rl_repo")

import numpy as np

import concourse.bacc as bacc
import concourse.mybir as mybir
import concourse.tile as tile
from concourse.bass_utils import run_bass_kernel_spmd
from concourse.masks import make_identity

S = 1024
D = 1024
H = 16
DH = 64
P = 128
KT = D // P  # 8
SBK = S // P  # 8
VW = H * (DH + 1)  # 1040
F32 = mybir.dt.float32
F16 = mybir.dt.float16

_CACHED_NC = None


def _build_nc():
    nc = bacc.Bacc("TRN2", target_bir_lowering=False, debug=False, num_devices=8)

    Qd = nc.dram_tensor("Qh", [S, D], F32, kind="ExternalInput")
    Kd = nc.dram_tensor("Kh", [S, D], F32, kind="ExternalInput")
    Vd = nc.dram_tensor("Vh", [S, D], F32, kind="ExternalInput")
    Wqd = nc.dram_tensor("Wq", [D, D], F32, kind="ExternalInput")
    Wkd = nc.dram_tensor("Wk", [D, D], F32, kind="ExternalInput")
    Wvd = nc.dram_tensor("Wv", [D, D], F32, kind="ExternalInput")
    Wod = nc.dram_tensor("Wo", [D, D], F32, kind="ExternalInput")
    bqd = nc.dram_tensor("bq", [D], F32, kind="ExternalInput")
    bkd = nc.dram_tensor("bk", [D], F32, kind="ExternalInput")
    attn_o = nc.dram_tensor("attn_o", [H, S, S], F32, kind="ExternalOutput")
    out_o = nc.dram_tensor("out_o", [S, D], F32, kind="ExternalOutput")

    with tile.TileContext(nc) as tc:
        _emit(nc, tc, Qd, Kd, Vd, Wqd, Wkd, Wvd, Wod, bqd, bkd, attn_o, out_o)

    if not nc.is_finalized():
        nc.finalize()
    return nc


def _transpose_into(nc, pools, Xd, XT_sb, ident16):
    """DMA-cast X [S,D] fp32->fp16, PE-transpose 128x128 blocks into
    XT_sb [128, KT*S] fp16 (k-tile t at free cols t*S + s)."""
    nat_pool, tps_pool = pools
    for sb_i in range(SBK):
        xnat = nat_pool.tile([P, D], F16, tag="xnat")
        nc.gpsimd.dma_start(out=xnat, in_=Xd[sb_i * P : (sb_i + 1) * P, :])
        for kb in range(KT):
            pt = tps_pool.tile([P, P], F16, tag="tps")
            nc.tensor.transpose(pt, xnat[:, kb * P : (kb + 1) * P], ident16)
            nc.vector.tensor_copy(
                XT_sb[:, kb * S + sb_i * P : kb * S + (sb_i + 1) * P], pt
            )


def _load_weight(nc, W_sb, Wd):
    for t in range(KT):
        nc.gpsimd.dma_start(
            out=W_sb[:, t * D : (t + 1) * D], in_=Wd[t * P : (t + 1) * P, :]
        )


def _emit(nc, tc, Qd, Kd, Vd, Wqd, Wkd, Wvd, Wod, bqd, bkd, attn_o, out_o):
    from contextlib import ExitStack

    with ExitStack() as top:
        consts = top.enter_context(tc.tile_pool(name="consts", bufs=1))
        ones_f = consts.tile([P, P], F32)
        nc.vector.memset(ones_f, 1.0)
        ident16 = consts.tile([P, P], F16)
        make_identity(nc, ident16)
        bq_sb = consts.tile([P, KT], F32)
        bk_sb = consts.tile([P, KT], F32)
        nc.sync.dma_start(out=bq_sb, in_=bqd[:].rearrange("(t p) -> p t", p=P))
        nc.sync.dma_start(out=bk_sb, in_=bkd[:].rearrange("(t p) -> p t", p=P))
        ones_col = consts.tile([1, P], F16)
        nc.vector.tensor_copy(ones_col, ones_f[0:1, :])
        ones_cols128 = consts.tile([P, H], F16)
        nc.vector.tensor_copy(ones_cols128, ones_f[:, 0:H])

        big = top.enter_context(tc.tile_pool(name="big", bufs=1))
        v_sb = big.tile([P, SBK * VW], F16, tag="v")
        qT_sb = big.tile([P, KT * S], F16, tag="qT")
        kT_sb = big.tile([P, KT * S], F16, tag="kT")
        outcT = big.tile([P, KT * S], F16, tag="outcT")
        Wo_sb = big.tile([P, KT * D], F16, tag="Wo")
        rcols = big.tile([P, H * SBK], F32, tag="rcols")  # col h*SBK+ib
        _load_weight(nc, Wo_sb, Wod)

        # ---- Prefix: V phase (VT + v), QT/KT transposes ----
        for sb_i in range(SBK):
            ones_dst = v_sb[:, sb_i * VW : (sb_i + 1) * VW].rearrange(
                "p (h c) -> p h c", c=DH + 1
            )[:, :, DH]
            nc.vector.tensor_copy(ones_dst, ones_cols128)
        with ExitStack() as ph:
            wp = ph.enter_context(tc.tile_pool(name="wv", bufs=1))
            xt = ph.enter_context(tc.tile_pool(name="vt", bufs=1))
            nat = ph.enter_context(tc.tile_pool(name="nat", bufs=3))
            tps = ph.enter_context(tc.tile_pool(name="tps", bufs=4, space="PSUM"))
            pps = ph.enter_context(tc.tile_pool(name="pps", bufs=2, space="PSUM"))
            Wv_sb = wp.tile([P, KT * D], F16, tag="wv")
            _load_weight(nc, Wv_sb, Wvd)
            Wq_sb = wp.tile([P, KT * D], F16, tag="wq")
            Wk_sb = wp.tile([P, KT * D], F16, tag="wk")
            _load_weight(nc, Wq_sb, Wqd)
            _load_weight(nc, Wk_sb, Wkd)
            VT_sb = xt.tile([P, KT * S], F16, tag="vt")
            QT_sb = xt.tile([P, KT * S], F16, tag="qt")
            KTr_sb = xt.tile([P, KT * S], F16, tag="kt")
            _transpose_into(nc, (nat, tps), Vd, VT_sb, ident16)
            for sb_i in range(SBK):
                pv = pps.tile([P, D], F32, tag="pp")
                for jn in range(2):
                    for t in range(KT):
                        nc.tensor.matmul(
                            pv[:, jn * 512 : (jn + 1) * 512],
                            VT_sb[:, t * S + sb_i * P : t * S + (sb_i + 1) * P],
                            Wv_sb[:, t * D + jn * 512 : t * D + (jn + 1) * 512],
                            start=(t == 0),
                            stop=(t == KT - 1),
                        )
                for h in range(H):
                    nc.vector.tensor_copy(
                        v_sb[
                            :,
                            sb_i * VW + h * (DH + 1) : sb_i * VW + h * (DH + 1) + DH,
                        ],
                        pv[:, h * DH : (h + 1) * DH],
                    )
            _transpose_into(nc, (nat, tps), Qd, QT_sb, ident16)
            _transpose_into(nc, (nat, tps), Kd, KTr_sb, ident16)
            for m in range(KT):
                for W_sb, XT, b_sb, dstT in (
                    (Wq_sb, QT_sb, bq_sb, qT_sb),
                    (Wk_sb, KTr_sb, bk_sb, kT_sb),
                ):
                    pq = pps.tile([P, S], F32, tag="pp", name=f"proj{m}")
                    for jn in range(2):
                        for t in range(KT):
                            nc.tensor.matmul(
                                pq[:, jn * 512 : (jn + 1) * 512],
                                W_sb[:, t * D + m * P : t * D + (m + 1) * P],
                                XT[:, t * S + jn * 512 : t * S + (jn + 1) * 512],
                                start=(t == 0),
                                stop=(t == KT - 1),
                            )
                    nc.vector.tensor_scalar_add(
                        dstT[:, m * S : (m + 1) * S], pq, b_sb[:, m : m + 1]
                    )

        # ---- Main pipeline: per head pair ----
        mp = top.enter_context(tc.tile_pool(name="mp", bufs=3, space="PSUM"))
        ops = top.enter_context(tc.tile_pool(name="ops", bufs=2, space="PSUM"))
        pexp = top.enter_context(tc.tile_pool(name="pexp", bufs=18))
        pbc = top.enter_context(tc.tile_pool(name="pbc", bufs=2))
        pattn = top.enter_context(tc.tile_pool(name="pattn", bufs=4))
        pout = top.enter_context(tc.tile_pool(name="pout", bufs=2))

        def emit_scores_exp(pr):
            # packed scoresT -> expT tiles (kept in SBUF for the PV pass)
            hc = pr * S
            ets = {}
            for jb in range(SBK):
                pst = [
                    mp.tile([P, S], F32, tag="sc", name=f"s2_{pr}_{jb}_{_u}")
                    for _u in range(2)
                ]
                for u in range(2):
                    hp = u * DH
                    for jn in range(2):
                        nc.tensor.matmul(
                            pst[u][:, jn * 512 : (jn + 1) * 512],
                            kT_sb[hp : hp + DH, hc + jb * P : hc + (jb + 1) * P],
                            qT_sb[hp : hp + DH, hc + jn * 512 : hc + (jn + 1) * 512],
                            start=True,
                            stop=True,
                            tile_position=(hp, 0),
                        )
                for u in range(2):
                    et = pexp.tile(
                        [P, S], F16, tag="expT", name=f"et{pr}_{jb}_{u}"
                    )
                    nc.scalar.activation(
                        et, pst[u], mybir.ActivationFunctionType.Exp, scale=0.125
                    )
                    ets[(u, jb)] = et
            return ets

        def emit_pv(pr, ets):
            # dense PV per (head, i-half): po [65,512] one PSUM bank each,
            # then immediately normalize that half of OutCatT + rcols
            hc = pr * S
            for u in range(2):
                h = 2 * pr + u
                hp = u * DH
                for jn in range(2):
                    po = ops.tile([DH + 1, 512], F32, tag="po", name=f"po{pr}{u}{jn}")
                    for jb in range(SBK):
                        nc.tensor.matmul(
                            po,
                            v_sb[
                                :,
                                jb * VW
                                + h * (DH + 1) : jb * VW
                                + (h + 1) * (DH + 1),
                            ],
                            ets[(u, jb)][:, jn * 512 : (jn + 1) * 512],
                            start=(jb == 0),
                            stop=(jb == SBK - 1),
                        )
                    rrf = pbc.tile([1, 512], F32, tag="rrf")
                    rscr = pbc.tile([1, 512], F32, tag="rscr")
                    nc.vector.reciprocal_approx_accurate(
                        rrf, po[DH : DH + 1, :], rscr
                    )
                    rr = pbc.tile([1, 512], F16, tag="rr")
                    nc.vector.tensor_copy(rr, rrf)
                    pb = mp.tile([DH, 512], F32, tag="sc", name=f"bc{pr}{u}{jn}")
                    nc.tensor.matmul(
                        pb, ones_col[0:1, 0:DH], rr, start=True, stop=True
                    )
                    bc = pbc.tile([DH, 512], F32, tag="bc")
                    nc.vector.tensor_copy(bc, pb)
                    nc.vector.tensor_mul(
                        outcT[
                            hp : hp + DH, hc + jn * 512 : hc + (jn + 1) * 512
                        ],
                        po[0:DH, :],
                        bc,
                    )
                    prc = mp.tile([P, 4], F32, tag="sc", name=f"rc{pr}{u}{jn}")
                    for b in range(4):
                        nc.tensor.transpose(
                            prc[:, b : b + 1],
                            rrf[0:1, b * P : (b + 1) * P],
                            ones_f[0:1, 0:1],
                        )
                    nc.vector.tensor_copy(
                        rcols[:, h * SBK + jn * 4 : h * SBK + jn * 4 + 4], prc
                    )

        def emit_stage3(pr):
            hc = pr * S
            for ib in range(SBK):
                pst = [
                    mp.tile([P, S], F32, tag="sc", name=f"s3_{pr}_{ib}_{_u}")
                    for _u in range(2)
                ]
                for u in range(2):
                    hp = u * DH
                    for jn in range(2):
                        nc.tensor.matmul(
                            pst[u][:, jn * 512 : (jn + 1) * 512],
                            qT_sb[hp : hp + DH, hc + ib * P : hc + (ib + 1) * P],
                            kT_sb[hp : hp + DH, hc + jn * 512 : hc + (jn + 1) * 512],
                            start=True,
                            stop=True,
                            tile_position=(hp, 0),
                        )
                for u in range(2):
                    h = 2 * pr + u
                    at = pattn.tile([P, S], F32, tag="attn")
                    nc.scalar.activation(
                        at, pst[u], mybir.ActivationFunctionType.Exp, scale=0.125
                    )
                    at2 = pattn.tile([P, S], F32, tag="attn2")
                    nc.vector.tensor_scalar_mul(
                        at2, at, rcols[:, h * SBK + ib : h * SBK + ib + 1]
                    )
                    nc.sync.dma_start(
                        out=attn_o[h, ib * P : (ib + 1) * P, :], in_=at2
                    )

        for pr in range(H // 2 + 1):
            ets = None
            if pr < H // 2:
                ets = emit_scores_exp(pr)
            if pr > 0:
                emit_stage3(pr - 1)
            if pr < H // 2:
                emit_pv(pr, ets)

        # ---- Stage IV: final projection ----
        for ib in range(SBK):
            pf = mp.tile([P, D], F32, tag="sc", name=f"fin{ib}")
            for jn in range(2):
                for t in range(KT):
                    nc.tensor.matmul(
                        pf[:, jn * 512 : (jn + 1) * 512],
                        outcT[:, t * S + ib * P : t * S + (ib + 1) * P],
                        Wo_sb[:, t * D + jn * 512 : t * D + (jn + 1) * 512],
                        start=(t == 0),
                        stop=(t == KT - 1),
                    )
            ot = pout.tile([P, D], F32, tag="ot")
            nc.vector.tensor_copy(ot, pf)
            nc.sync.dma_start(out=out_o[ib * P : (ib + 1) * P, :], in_=ot)


def _get_nc():
    global _CACHED_NC
    if _CACHED_NC is None:
        _CACHED_NC = _build_nc()
    return _CACHED_NC


def kernel(Q, K, V, mask, Wq, bq, Wk, bk, Wv, bv, Wo, bo, _want_results=False):
    Q = np.ascontiguousarray(np.asarray(Q, dtype=np.float32))
    K = np.ascontiguousarray(np.asarray(K, dtype=np.float32))
    V = np.ascontiguousarray(np.asarray(V, dtype=np.float32))
    Wq = np.ascontiguousarray(np.asarray(Wq, dtype=np.float32))
    Wk = np.ascontiguousarray(np.asarray(Wk, dtype=np.float32))
    Wv = np.ascontiguousarray(np.asarray(Wv, dtype=np.float32))
    Wo = np.ascontiguousarray(np.asarray(Wo, dtype=np.float32))
    bq = np.ascontiguousarray(np.asarray(bq, dtype=np.float32))
    bk = np.ascontiguousarray(np.asarray(bk, dtype=np.float32))
    bv = np.asarray(bv, dtype=np.float32)
    bo = np.asarray(bo, dtype=np.float32)
    mask = np.asarray(mask, dtype=np.float32)
    assert not np.any(mask), "kernel assumes zero mask (input spec fill=zeros)"

    nc = _get_nc()
    B = Q.shape[0]
    in_maps = [
        {
            "Qh": Q[c],
            "Kh": K[c],
            "Vh": V[c],
            "Wq": Wq,
            "Wk": Wk,
            "Wv": Wv,
            "Wo": Wo,
            "bq": bq,
            "bk": bk,
        }
        for c in range(B)
    ]
    res = run_bass_kernel_spmd(nc, in_maps, core_ids=list(range(B)))

    out = np.stack([res.results[c]["out_o"] for c in range(B)])
    attn = np.stack([res.results[c]["attn_o"] for c in range(B)])
    out += (bv @ Wo + bo)[None, None, :]
    if _want_results:
        return (out, attn), res
    return out, attn
